# revision 17
# baseline (speedup 1.0000x reference)
"""Trainium2 Bass kernel for the AdaptiveSNN problem.

Strategy (pure data parallelism across 8 NeuronCores, batch 16384 -> 2048/core):
  - host: transpose x/W1 so the contraction dim (784) lands on SBUF partitions
  - cur1 = W1 @ x^T + b1 computed once per core in fp32 on the TensorEngine
    (layout [128 neurons, 2048 batch])
  - 25 sequential LIF steps; the whole membrane update
        m' = (beta*m + cur) - (m > 1)
    is ONE custom DVE instruction (registered below). Layer-1 spikes for the
    layer-2 matmul come from the ScalarEngine as Sign(m-1) in bf16 (+-1 exact).
  - layer-2 matmul per 128-batch chunk: stationary = sign chunk (bf16),
    moving = 0.5*W2^T split hi/lo in bf16 (exact to ~1e-7), plus a K=1
    fp32 ones x cc matmul that adds the constant 0.5*sum(W2)+b2 row.
    Output lands batch-major [128, 16*10] so layer-2 elementwise is cheap.
  - outputs DMA'd per step as [25, 128, 160] per core; host reassembles.
"""
import numpy as np
import ml_dtypes

import concourse.bacc as bacc
import concourse.mybir as mybir
import concourse.tile as tile
from concourse.tile import add_dep_helper
import concourse.dve_ops as dve_ops
from concourse.dve_spec import Spec, Src0, Src1, C0, C1
from concourse.dve_ops import DveOp
from concourse.bass_utils import run_bass_kernel_spmd

F32 = mybir.dt.float32
BF16 = mybir.dt.bfloat16

N_CORES = 8
B_FULL = 16384
B = B_FULL // N_CORES          # 2048 batch rows per core
D_IN = 784                     # 28*28
H1 = 128
H2 = 10
STEPS = 25
KT = 112                       # K-tile size: 784 = 7 * 112
NKT = D_IN // KT
NCHUNK = B // 128              # 16 batch chunks per core
THRESH = 1.0


def _register_lif():
    """Custom DVE op: out = ((in0*s0 + in1) - (in0 > s1)).

    Matches the reference's fp32 association order exactly:
    new_mem = (beta*mem + cur) - reset.
    """
    if "LIF_STEP_ANT" in dve_ops._SUB_OPCODE_FOR_NAME:
        return next(op for op in dve_ops.OPS if op.name == "LIF_STEP_ANT")
    op = DveOp(
        "LIF_STEP_ANT",
        Spec(
            body=(Src0 * C0 + Src1) - (Src0 > C1),
            reference=lambda in0, in1, s0, s1, imm2: (
                (in0 * s0 + in1) - (in0 > s1).astype(np.float32)
            ),
        ),
        subdim=False,
        uops_sha={"v3": "4d971942aba05d49", "v4": "da6677450a1cb1b9"},
    )
    dve_ops.OPS.append(op)
    dve_ops._SUB_OPCODE_FOR_NAME[op.name] = (
        dve_ops._CUSTOM_DVE_ROW_BASE + len(dve_ops.OPS) - 1
    )
    dve_ops.CUSTOM_DVE_SPECS[op.name] = op.spec
    return op


_GRAPH_CACHE = {}


def _build_graph(beta1: float, beta2: float):
    key = (beta1, beta2)
    if key in _GRAPH_CACHE:
        return _GRAPH_CACHE[key]
    LIF = _register_lif()
    Sign = mybir.ActivationFunctionType.Sign

    nc = bacc.Bacc("TRN2", target_bir_lowering=False, debug=False,
                   num_devices=N_CORES)

    xt_d = nc.dram_tensor("xt", [NKT, KT, B], F32, kind="ExternalInput").ap()
    w1t_d = nc.dram_tensor("w1t", [KT, NKT * H1], F32, kind="ExternalInput").ap()
    b1_d = nc.dram_tensor("b1", [H1, 1], F32, kind="ExternalInput").ap()
    w2h_d = nc.dram_tensor("w2h", [H1, H2], BF16, kind="ExternalInput").ap()
    w2l_d = nc.dram_tensor("w2l", [H1, H2], BF16, kind="ExternalInput").ap()
    cch_d = nc.dram_tensor("cch", [1, NCHUNK * H2], BF16, kind="ExternalInput").ap()
    ccl_d = nc.dram_tensor("ccl", [1, NCHUNK * H2], BF16, kind="ExternalInput").ap()

    out_mem = nc.dram_tensor("out_mem", [STEPS, 128, NCHUNK * H2], F32,
                             kind="ExternalOutput").ap()

    with tile.TileContext(nc) as tc:
        with tc.tile_pool(name="const", bufs=1) as cpool, \
             tc.tile_pool(name="xin", bufs=1) as xpool, \
             tc.tile_pool(name="m1p", bufs=2) as m1pool, \
             tc.tile_pool(name="m2p", bufs=3) as m2pool, \
             tc.tile_pool(name="s2p", bufs=3) as s2pool, \
             tc.tile_pool(name="sgp", bufs=3) as sgpool, \
             tc.tile_pool(name="ps1", bufs=1, space="PSUM") as ps1pool, \
             tc.tile_pool(name="ps2", bufs=3, space="PSUM") as ps2pool:

            # preload the ACT function tables before anything else
            warm_t = cpool.tile([H1, 1], F32, tag="warm")
            nc.scalar.activation(warm_t[:], nc.const_aps.tensor(0.0, (H1, 1)),
                                 Sign, bias=0.0)
            # ---- load constants first (ahead of the big chained xt DMAs
            # in the queue FIFOs), then the input shard ----
            b1_t = cpool.tile([H1, 1], F32, tag="b1")
            nc.sync.dma_start(b1_t[:], b1_d)
            w2h_t = cpool.tile([H1, H2], BF16, tag="w2h")
            nc.sync.dma_start(w2h_t[:], w2h_d)
            w2l_t = cpool.tile([H1, H2], BF16, tag="w2l")
            nc.sync.dma_start(w2l_t[:], w2l_d)
            cch_t = cpool.tile([1, NCHUNK * H2], BF16, tag="cch")
            nc.sync.dma_start(cch_t[:], cch_d)
            ccl_t = cpool.tile([1, NCHUNK * H2], BF16, tag="ccl")
            nc.sync.dma_start(ccl_t[:], ccl_d)
            ones_t = cpool.tile([1, H1], BF16, tag="ones")
            nc.vector.memset(ones_t[:], 1.0)
            w1t_all = cpool.tile([KT, NKT * H1], F32, tag="w1t")
            nc.sync.dma_start(w1t_all[:], w1t_d)
            w1t_tiles = [w1t_all[:, k * H1:(k + 1) * H1] for k in range(NKT)]
            neg1_t = cpool.tile([H1, 1], F32, tag="neg1")
            nc.vector.memset(neg1_t[:], -1.0)
            # Chain xt DMAs with a 2-K in-flight window: the HW queue
            # round-robins packets across all outstanding transfers, which
            # would make every K-slice arrive simultaneously at the end;
            # chaining gives sequential arrival so matmuls pipeline behind.
            Q = B // 4
            xt_tiles = []
            for k in range(NKT):
                xt = xpool.tile([KT, B], F32, tag=f"xt{k}", name=f"xt{k}")
                xt_tiles.append(xt)
            prev_last = None
            for q in range(4):
                last = None
                for k in range(NKT):
                    dq = nc.sync.dma_start(xt_tiles[k][:, q * Q:(q + 1) * Q],
                                           xt_d[k][:, q * Q:(q + 1) * Q])
                    if prev_last is not None:
                        add_dep_helper(dq.ins, prev_last.ins, sync=True,
                                       reason="stage col-quarter arrival")
                    last = dq
                prev_last = last

            # ---- cur1 = W1 @ x^T + b1 : [128 neurons, 2048 batch] ----
            cur1_t = cpool.tile([H1, B], F32, tag="cur1")
            NT = B // 512
            ps1_tiles = [ps1pool.tile([H1, 512], F32, tag=f"ps1_{nt}",
                                      name=f"ps1_{nt}") for nt in range(NT)]
            for nt in range(NT):
                for k in range(NKT):
                    nc.tensor.matmul(
                        ps1_tiles[nt][:], w1t_tiles[k],
                        xt_tiles[k][:, nt * 512:(nt + 1) * 512],
                        start=(k == 0), stop=(k == NKT - 1))
                # psum -> sbuf with +b1 per-partition bias on the VectorEngine
                nc.vector.tensor_scalar(cur1_t[:, nt * 512:(nt + 1) * 512],
                                        ps1_tiles[nt][:], b1_t[:], None,
                                        mybir.AluOpType.add)

            # ---- states ----
            m2 = m2pool.tile([128, NCHUNK * H2], F32, tag="m2")
            nc.vector.memset(m2[:], 0.0)

            # ---- time loop ----
            # Software-pipelined by one step: step t's layer-2 DVE work is
            # emitted AFTER step t+1's layer-1 LIF, so the DVE never stalls
            # waiting for the ScalarE(sign) -> TensorE(matmul) chain.
            def finish_l2(ps2, t, m2_cur):
                m2n = m2pool.tile([128, NCHUNK * H2], F32, tag="m2")
                nc.vector._custom_dve(LIF, out=m2n[:], in0=m2_cur[:],
                                      in1=ps2[:], s0=beta2, s1=THRESH)
                nc.sync.dma_start(out_mem[t], m2n[:])
                return m2n

            # Lag-2 software pipeline: step t's layer-2 membrane update is
            # emitted after step t+2's layer-1 LIF, so the DVE stream never
            # waits on the ScalarE(sign) -> TensorE(matmul) chain.
            pend = []  # [(psum2, t), ...] steps whose layer-2 is unfinished
            m1 = cur1_t  # step 0: m1' = (0.9*0 + cur1) - (0>1) = cur1
            for t in range(STEPS):
                if t > 0:
                    # layer-1 membrane update: one DVE instruction
                    m1n = m1pool.tile([H1, B], F32, tag="m1")
                    nc.vector._custom_dve(LIF, out=m1n[:], in0=m1[:],
                                          in1=cur1_t[:], s0=beta1, s1=THRESH)
                    m1 = m1n
                if len(pend) >= 2:
                    m2 = finish_l2(*pend.pop(0), m2)
                # spikes as signs (bf16, +-1) on the ScalarEngine
                sg = sgpool.tile([H1, B], BF16, tag="sg")
                nc.scalar.activation(sg[:], m1[:], Sign, bias=neg1_t[:])
                # layer-2: cur2 = cc + 0.5*W2@sg  (batch-major [128, 160])
                # cc lands via two K=1 bf16 broadcast matmuls over all cols
                ps2 = ps2pool.tile([128, NCHUNK * H2], F32, tag="ps2")
                nc.tensor.matmul(ps2[:], ones_t[:], cch_t[:], start=True,
                                 stop=False)
                nc.tensor.matmul(ps2[:], ones_t[:], ccl_t[:], start=False,
                                 stop=False)
                for c in range(NCHUNK):
                    o = ps2[:, c * H2:(c + 1) * H2]
                    sgc = sg[:, c * 128:(c + 1) * 128]
                    nc.tensor.matmul(o, sgc, w2h_t[:], start=False, stop=False)
                    nc.tensor.matmul(o, sgc, w2l_t[:],
                                     start=False, stop=(c == NCHUNK - 1))
                pend.append((ps2, t))
            while pend:
                m2 = finish_l2(*pend.pop(0), m2)

    nc.compile()
    _GRAPH_CACHE[key] = nc
    return nc


def prepare_in_maps(x, W1, b1, W2, b2):
    x = np.asarray(x, dtype=np.float32)
    W1 = np.asarray(W1, dtype=np.float32)
    b1 = np.asarray(b1, dtype=np.float32)
    W2 = np.asarray(W2, dtype=np.float32)
    b2 = np.asarray(b2, dtype=np.float32)
    xf = x.reshape(B_FULL, D_IN)
    xT = xf.T.reshape(NKT, KT, B_FULL)                    # [7, 112, 16384]
    W1T = np.ascontiguousarray(
        W1.T.reshape(NKT, KT, H1).transpose(1, 0, 2).reshape(KT, NKT * H1))
    b1c = np.ascontiguousarray(b1.reshape(H1, 1))
    W2T_half = 0.5 * W2.T                                 # [128, 10]
    w2h = W2T_half.astype(ml_dtypes.bfloat16)
    w2l = (W2T_half - w2h.astype(np.float32)).astype(ml_dtypes.bfloat16)
    ccrow = (0.5 * W2.sum(axis=1) + b2).astype(np.float32)
    cc160f = np.tile(ccrow, NCHUNK).reshape(1, NCHUNK * H2)
    cch = cc160f.astype(ml_dtypes.bfloat16)
    ccl = (cc160f - cch.astype(np.float32)).astype(ml_dtypes.bfloat16)
    in_maps = []
    for i in range(N_CORES):
        shard = np.ascontiguousarray(xT[:, :, i * B:(i + 1) * B])
        in_maps.append({
            "xt": shard, "w1t": W1T, "b1": b1c,
            "w2h": w2h, "w2l": w2l, "cch": cch, "ccl": ccl,
        })
    return in_maps


def kernel(x, W1, b1, W2, b2, beta1, beta2):
    bb1 = float(np.clip(np.float32(beta1), 0.0, 1.0))
    bb2 = float(np.clip(np.float32(beta2), 0.0, 1.0))
    in_maps = prepare_in_maps(x, W1, b1, W2, b2)
    nc = _build_graph(bb1, bb2)
    res = run_bass_kernel_spmd(nc, in_maps, list(range(N_CORES)), trace=False)

    mem_parts = []
    for i in range(N_CORES):
        r = res.results[i]
        # [25, 128, 16*10] -> [25, 2048, 10]: batch = chunk*128 + partition
        mem = r["out_mem"].reshape(STEPS, 128, NCHUNK, H2)
        mem_parts.append(np.transpose(mem, (0, 2, 1, 3)).reshape(STEPS, B, H2))
    mem2 = np.ascontiguousarray(
        np.concatenate(mem_parts, axis=1).astype(np.float32))
    # spikes are a pure function of the (bit-exact) membrane values
    spk2 = (mem2 > np.float32(THRESH)).astype(np.float32)
    return spk2, mem2
